# revision 23
# baseline (speedup 1.0000x reference)
"""3-layer GAT (8 heads x 32 hid, PyG GATConv semantics w/ self-loops) +
global mean pool + linear, distributed over 8 Trainium2 NeuronCores.

Strategy (per sharding hint): nodes partitioned into 8 contiguous ranges
(dst-owner); incident edges live with their dst core, sorted by dst then src.
Each layer: node phase computes hh=[h|alpha_src] and alpha_dst for local
nodes (bf16), AllGather replicates hh; edge phase gathers hh[src] rows
(SWDGE dma_gather, up to KB 128-edge tiles per instruction, bf16 rows),
computes un-normalized attention numerator and denominator with one-hot
segment matmuls (bf16) accumulated in PSUM, then divides per dst node
(segment softmax folded: out = sum(ex*h)/sum(ex); e is O(1)-bounded so no
max subtraction). The ELU "-1" offset is folded into the next layer's
weights (corr row) / the final linear bias. The next layer's node tiles
are software-pipelined into the edge phase (SBUF hand-off), with the hh
AllGather split into two chunks (int16-index-limit sized) that overlap
remaining edge work. The mean-pool result is reduced to a per-graph
scalar locally, so only a [512,1] vector is AllReduced at the end.

Self-contained: hardcodes the problem shapes; host-side preprocessing uses
only graph structure (edge_index, batch) and parameter repacking.
"""
import math
import numpy as np
import ml_dtypes

import concourse.bass as bass
import concourse.bacc as bacc
import concourse.mybir as mybir
import concourse.tile as tile

P = 128
KB = 8                    # dst tiles gathered per SWDGE dma_gather
WORK_BUFS = 6
GATHER_BUFS = 6
HEADS, HID = 8, 32
DH = HEADS * HID          # 256
DA = DH + HEADS           # 264 = h | alpha_src
DW = DH + 2 * HEADS       # 272 = W | W@Asrc | W@Adst
DG = 384                  # gathered bf16 row: DG*2B % 256B == 0
IN_CH = 128
NEG = 0.2
F32 = mybir.dt.float32
BF16 = mybir.dt.bfloat16
I32 = mybir.dt.int32
BF = ml_dtypes.bfloat16


# ----------------------------------------------------------------- host prep

def host_prep(x, edge_index, batch, Ws, a_srcs, a_dsts, biases, lin_w, lin_b,
              N, E, G, ncores):
    """Build per-core input maps + the (core-uniform) program config.

    Edge layout: per dst tile, edges are split by src half-table (int16
    index limit of dma_gather), each half padded to whole 128-edge tiles;
    tiles are gathered in groups of <=KB via one dma_gather each.
    """
    nl_real = N // ncores
    assert nl_real * ncores == N
    NL = ((nl_real + P - 1) // P) * P          # padded local nodes
    NT = NL // P                               # dst tiles per core
    GP = ((G + P - 1) // P) * P                # padded graphs
    NG = NL * ncores
    CHA = 4096                 # rows/core in AllGather chunk A (lo table)
    CHB = NL - CHA             # rows/core in chunk B (hi table)
    assert ncores * CHA <= 32768 and ncores * CHB <= 32768

    src = np.concatenate([edge_index[0], np.arange(N, dtype=np.int64)])
    dst = np.concatenate([edge_index[1], np.arange(N, dtype=np.int64)])
    core_of = dst // nl_real
    dloc_all = dst - core_of * nl_real

    # per-core, per-tile, per-half edge lists sorted by (dst_local, src)
    per_core = []
    clo = np.zeros((ncores, NT), np.int64)
    chi = np.zeros((ncores, NT), np.int64)
    for k in range(ncores):
        m = core_of == k
        s_k, d_k = src[m], dloc_all[m]
        ks, rs = s_k // nl_real, s_k % nl_real
        lo = rs < CHA
        # row in the lo table (chunk A) or hi table (chunk B)
        gsrc = np.where(lo, ks * CHA + rs, ks * CHB + (rs - CHA))
        order = np.lexsort((gsrc, d_k))
        gsrc, d_k, lo = gsrc[order], d_k[order], lo[order]
        t_k = d_k // P
        tiles = []
        for t in range(NT):
            mt = t_k == t
            g_t, d_t, lo_t = gsrc[mt], d_k[mt] - t * P, lo[mt]
            glo, dlo = g_t[lo_t], d_t[lo_t]
            ghi, dhi = g_t[~lo_t], d_t[~lo_t]
            if t == NT - 1 and NL > nl_real:
                npad = NL - nl_real
                glo = np.concatenate([glo, np.zeros(npad, np.int64)])
                dlo = np.concatenate([dlo, np.arange(nl_real - t * P,
                                                     nl_real - t * P + npad)])
            tiles.append(((glo, dlo), (ghi, dhi)))
            clo[k, t] = len(glo)
            chi[k, t] = len(ghi)
        per_core.append(tiles)
    m_lo = [int(v) for v in np.maximum(np.ceil(clo.max(axis=0) / P), 1).astype(np.int64)]
    m_hi = [int(v) for v in np.ceil(chi.max(axis=0) / P).astype(np.int64)]
    SM = int(sum(m_lo) + sum(m_hi))
    # column offset of each tile's run (lo tiles then hi tiles)
    col0 = []
    c = 0
    for t in range(NT):
        col0.append(c)
        c += m_lo[t] + m_hi[t]

    in_maps = []
    batch = np.asarray(batch, np.int64)
    for k in range(ncores):
        dstflat = np.full((P, SM), 999.0, np.float32)
        idx16 = np.zeros((16, 8 * SM), np.int16)
        for t in range(NT):
            for half, mh in ((0, m_lo[t]), (1, m_hi[t])):
                if mh == 0:
                    continue
                g_t, d_t = per_core[k][t][half]
                L = mh * P
                gi = np.zeros(L, np.int64)
                gi[:len(g_t)] = g_t
                dd = np.full(L, 999.0, np.float32)
                dd[:len(d_t)] = d_t
                c0 = col0[t] + (m_lo[t] if half else 0)
                ii = np.arange(L)
                dstflat[ii % P, c0 + ii // P] = dd
                # groups of <=KB tiles; idx wrap is per group
                j = 0
                while j < mh:
                    kb = min(KB, mh - j)
                    arr = gi[j * P:(j + kb) * P]
                    idx16[:, (c0 + j) * 8:(c0 + j + kb) * 8] = \
                        arr.reshape(-1, 16).T.astype(np.int16)
                    j += kb

        bl = batch[k * nl_real:(k + 1) * nl_real]
        g_base = int(bl[0])
        bshift = np.full(NL, 999.0, np.float32)
        bshift[:nl_real] = (bl - g_base).astype(np.float32)
        gslot = np.arange(g_base, g_base + P, dtype=np.int64)
        gslot = np.where(gslot < G, gslot, 99999).astype(np.int32)[:, None]

        xk = np.zeros((NL, IN_CH), np.float32)
        xk[:nl_real] = x[k * nl_real:(k + 1) * nl_real]

        im = {
            "x_local": xk,
            "idx16": np.tile(idx16, (8, 1)),
            "dstloc": dstflat.reshape(-1).astype(BF),
            "bshift": bshift,
            "gslot": gslot,
        }
        in_maps.append(im)

    # replicated parameters
    def aug(W, a_s, a_d):
        As = np.zeros((DH, HEADS), np.float32)
        Ad = np.zeros((DH, HEADS), np.float32)
        for h in range(HEADS):
            As[h * HID:(h + 1) * HID, h] = a_s[h]
            Ad[h * HID:(h + 1) * HID, h] = a_d[h]
        return np.concatenate([W, W @ As, W @ Ad], axis=1).astype(np.float32)

    Waugs = [aug(Ws[l], a_srcs[l], a_dsts[l]) for l in range(3)]
    Waugs_bf = [w.astype(BF) for w in Waugs]
    # ELU writes out' = elu(x)+1; next layer corrects (out'-1)@W = out'@W-corr
    corrs = [np.tile(Waugs_bf[l].astype(np.float32).sum(axis=0)[None, :],
                     (P, 1)).astype(np.float32) for l in (1, 2)]
    cnts = np.bincount(batch, minlength=G).astype(np.float32)
    invcnt = np.zeros((GP, 1), np.float32)
    invcnt[:G, 0] = 1.0 / np.maximum(cnts, 1.0)
    lin_b_fold = np.float32(lin_b.reshape(-1)[0] - lin_w.sum())
    params = {
        "W0aug": Waugs_bf[0],
        "W1aug": Waugs_bf[1],
        "W2aug": Waugs_bf[2],
        "corr1": corrs[0],
        "corr2": corrs[1],
        "b0": np.tile(biases[0][None, :], (P, 1)).astype(np.float32),
        "b1": np.tile(biases[1][None, :], (P, 1)).astype(np.float32),
        "b2": np.tile(biases[2][None, :], (P, 1)).astype(np.float32),
        "linw": np.tile(lin_w.reshape(1, DH), (P, 1)).astype(np.float32),
        "linb": np.full((P, 1), lin_b_fold, np.float32),
        "invcnt": invcnt,
    }
    for im in in_maps:
        im.update(params)

    cfg = dict(NL=NL, NT=NT, GP=GP, m_lo=tuple(m_lo), m_hi=tuple(m_hi),
               SM=SM, ncores=ncores)
    return cfg, in_maps


# ------------------------------------------------------------- program build

def build_program(cfg, use_f32r=False, reps=1, dup=None, no_cc=False):
    NL, NT, GP = cfg["NL"], cfg["NT"], cfg["GP"]
    SM, ncores = cfg["SM"], cfg["ncores"]
    m_lo, m_hi = cfg["m_lo"], cfg["m_hi"]
    NG = NL * ncores                     # padded-global node rows
    CHA = 4096                           # chunk A rows/core (lo table)
    CHB = NL - CHA                       # chunk B rows/core (hi table)
    TA = CHA // P                        # node tiles in chunk A
    GLA = ncores * CHA                   # global rows in lo table

    nc = bacc.Bacc("TRN2", target_bir_lowering=False, debug=False,
                   num_devices=ncores, dynamic_dma_scratch_size=32768)
    # ---------------- I/O
    x_in = nc.dram_tensor("x_local", [NL, IN_CH], F32, kind="ExternalInput")
    idx16 = nc.dram_tensor("idx16", [P, 8 * SM], mybir.dt.int16,
                           kind="ExternalInput")
    dstloc = nc.dram_tensor("dstloc", [P * SM], BF16, kind="ExternalInput")
    bshift = nc.dram_tensor("bshift", [NL], F32, kind="ExternalInput")
    gslot = nc.dram_tensor("gslot", [P, 1], I32, kind="ExternalInput")
    Waug = [nc.dram_tensor(f"W{l}aug", [IN_CH if l == 0 else DH, DW], BF16,
                           kind="ExternalInput") for l in range(3)]
    corr_d = [nc.dram_tensor(f"corr{l}", [P, DW], F32, kind="ExternalInput")
              for l in (1, 2)]
    bias = [nc.dram_tensor(f"b{l}", [P, DH], F32, kind="ExternalInput")
            for l in range(3)]
    linw = nc.dram_tensor("linw", [P, DH], F32, kind="ExternalInput")
    linb = nc.dram_tensor("linb", [P, 1], F32, kind="ExternalInput")
    invcnt = nc.dram_tensor("invcnt", [GP, 1], F32, kind="ExternalInput")
    y = nc.dram_tensor("y", [GP, 1], F32, kind="ExternalOutput")

    with tile.TileContext(nc) as tc:
        with tc.tile_pool(name="const", bufs=1) as cst, \
             tc.tile_pool(name="dram", bufs=1, space="DRAM") as dram, \
             tc.tile_pool(name="work", bufs=WORK_BUFS) as wk, \
             tc.tile_pool(name="work2", bufs=4) as wk2, \
             tc.tile_pool(name="gpool", bufs=GATHER_BUFS) as gp, \
             tc.tile_pool(name="psA", bufs=2, space="PSUM") as psA, \
             tc.tile_pool(name="psB", bufs=1, space="PSUM") as psB, \
             tc.tile_pool(name="psC", bufs=2, space="PSUM") as psC, \
             tc.tile_pool(name="psN", bufs=2, space="PSUM") as psN, \
             tc.tile_pool(name="psP", bufs=1, space="PSUM") as psP:

            # ---------------- DRAM intermediates (lo/hi split so the two
            # AllGather chunks and their consumers have independent deps)
            hh_loc_a = dram.tile([CHA, DG], BF16, tag="hhla", name="hhla")
            hh_loc_b = dram.tile([CHB, DG], BF16, tag="hhlb", name="hhlb")
            ad_local = dram.tile([NL, HEADS], BF16)
            hh_lo_b = [dram.tile([GLA, DG], BF16, tag=f"hhlo{i}",
                                 name=f"hhlo{i}") for i in range(2)]
            hh_hi_b = [dram.tile([NG - GLA, DG], BF16, tag=f"hhhi{i}",
                                 name=f"hhhi{i}") for i in range(2)]
            z_loc = dram.tile([GP, 1], F32)
            z_sum = dram.tile([GP, 1], F32)

            # ---------------- constants
            ident = cst.tile([P, P], BF16)     # transpose identity (bf16)
            iota_i = cst.tile([P, P], I32)
            nc.gpsimd.iota(iota_i[:], pattern=[[1, P]], base=0,
                           channel_multiplier=0)
            iota_f = cst.tile([P, P], F32)
            nc.vector.tensor_copy(iota_f[:], iota_i[:])
            iota_b = cst.tile([P, P], BF16)
            nc.vector.tensor_copy(iota_b[:], iota_i[:])
            iota_ci = cst.tile([P, 1], I32)
            nc.gpsimd.iota(iota_ci[:], pattern=[[0, 1]], base=0,
                           channel_multiplier=1)
            iota_cf = cst.tile([P, 1], F32)
            nc.vector.tensor_copy(iota_cf[:], iota_ci[:])
            nc.vector.tensor_tensor(out=ident[:],
                                    in0=iota_cf[:].to_broadcast([P, P]),
                                    in1=iota_f[:], op=mybir.AluOpType.is_equal)
            zero_dh = cst.tile([P, DH], F32)
            nc.gpsimd.memset(zero_dh[:], 0.0)
            iota_rep = cst.tile([P, KB * P], BF16)
            for q in range(KB):
                nc.vector.tensor_copy(iota_rep[:, q * P:(q + 1) * P], iota_b[:])

            idx_all = cst.tile([P, 8 * SM], mybir.dt.int16)
            nc.sync.dma_start(idx_all[:], idx16[:, :])
            dst_all = cst.tile([P, SM], BF16)
            nc.sync.dma_start(dst_all[:], dstloc[:].rearrange("(p j) -> p j", j=SM))

            W_t = []
            for l in range(3):
                cin = IN_CH if l == 0 else DH
                tiles = []
                for kk in range(cin // P):
                    t = cst.tile([P, DW], BF16, tag=f"W{l}_{kk}")
                    nc.sync.dma_start(t[:], Waug[l][kk * P:(kk + 1) * P, :])
                    tiles.append(t)
                W_t.append(tiles)
            corr_t = {}
            for i, l in enumerate((1, 2)):
                t = cst.tile([P, DW], F32, tag=f"corr{l}")
                nc.sync.dma_start(t[:], corr_d[i][:, :])
                corr_t[l] = t
            bias_t = []
            for l in range(3):
                t = cst.tile([P, DH], F32, tag=f"bias{l}")
                nc.sync.dma_start(t[:], bias[l][:, :])
                bias_t.append(t)
            linw_t = cst.tile([P, DH], F32)
            nc.sync.dma_start(linw_t[:], linw[:, :])
            linb_t = cst.tile([P, 1], F32)
            nc.sync.dma_start(linb_t[:], linb[:, :])
            gslot_t = cst.tile([P, 1], I32)
            nc.sync.dma_start(gslot_t[:], gslot[:, :])
            # hh_local pad columns are never written by the node phase but are
            # AllGathered; zero them once so sim stays finite.
            zpad = cst.tile([P, DG - DA], BF16)
            nc.gpsimd.memset(zpad[:], 0.0)
            for nt in range(NT):
                if nt < TA:
                    nc.sync.dma_start(
                        hh_loc_a[nt * P:(nt + 1) * P, DA:DG], zpad[:])
                else:
                    r = nt * P - CHA
                    nc.sync.dma_start(hh_loc_b[r:r + P, DA:DG], zpad[:])

            # ---------------- phases
            def node_tile(l, nt, src_tile):
                """One 128-row tile of h_in @ Waug_l -> hh_local, ad_local.

                src_tile: bf16 [P, cin] SBUF tile (edge-phase output), or
                None for layer 0 (loads x from DRAM and casts). helu holds
                elu+1; corr_t[l] (= 1_row @ Waug_l) subtracts the offset.
                """
                cin = IN_CH if l == 0 else DH
                if src_tile is None:
                    in_f = wk2.tile([P, IN_CH], F32, tag="node_inf")
                    nc.sync.dma_start(in_f[:], x_in[nt * P:(nt + 1) * P, :])
                    in_b = wk2.tile([P, IN_CH], BF16, tag="node_inb0")
                    nc.vector.tensor_copy(in_b[:], in_f[:])
                else:
                    in_b = src_tile
                ps_o = psN.tile([P, DW], F32, space="PSUM", tag="node_mm")
                for kk in range(cin // P):
                    trp = psC.tile([P, P], BF16, space="PSUM", tag="trpb")
                    nc.tensor.transpose(out=trp[:],
                                        in_=in_b[:, kk * P:(kk + 1) * P],
                                        identity=ident[:])
                    inT = wk2.tile([P, P], BF16, tag="node_inT")
                    nc.vector.tensor_copy(inT[:], trp[:])
                    nc.tensor.matmul(ps_o[:], lhsT=inT[:], rhs=W_t[l][kk][:],
                                     start=(kk == 0), stop=(kk == cin // P - 1))
                hh_t = wk2.tile([P, DW], F32, tag="node_hh")
                if l == 0:
                    nc.vector.tensor_copy(hh_t[:], ps_o[:])
                else:
                    nc.vector.tensor_sub(hh_t[:], ps_o[:], corr_t[l][:])
                hh_b = wk2.tile([P, DA], BF16, tag="node_hhb")
                nc.vector.tensor_copy(hh_b[:], hh_t[:, 0:DA])
                ad_b = wk2.tile([P, HEADS], BF16, tag="node_adb")
                nc.vector.tensor_copy(ad_b[:], hh_t[:, DA:DW])
                if nt < TA:
                    nc.sync.dma_start(hh_loc_a[nt * P:(nt + 1) * P, 0:DA],
                                      hh_b[:])
                else:
                    r = nt * P - CHA
                    nc.sync.dma_start(hh_loc_b[r:r + P, 0:DA], hh_b[:])
                nc.sync.dma_start(ad_local[nt * P:(nt + 1) * P, :], ad_b[:])

            def ag_chunk(l, chunk):
                """AllGather one hh chunk into layer-l's lo/hi table."""
                if chunk == 0:
                    ins_ap, outs_ap = hh_loc_a[:, :], hh_lo_b[l % 2][:, :]
                else:
                    ins_ap, outs_ap = hh_loc_b[:, :], hh_hi_b[l % 2][:, :]
                if no_cc:
                    nc.sync.dma_start(outs_ap.tensor[0:(CHA if chunk == 0
                                                         else CHB), :], ins_ap)
                    return
                nc.gpsimd.collective_compute(
                    "AllGather", mybir.AluOpType.bypass,
                    ins=[ins_ap.opt()], outs=[outs_ap.opt()],
                    replica_groups=[list(range(ncores))])

            def edge_phase(l):
                """Per dst tile: gather, attention, segment-matmul, epilogue.

                For l<2 the next layer's node tile is issued right after each
                epilogue (SBUF hand-off), and the next layer's AllGather
                chunks fire after node tiles TA-1 / NT-1 so the collective
                overlaps the remaining edge work.
                """
                last = (l == 2)
                hh_lo, hh_hi = hh_lo_b[l % 2], hh_hi_b[l % 2]
                if last:
                    pool_ps = psP.tile([P, DH], F32, space="PSUM", tag="pool")
                off = 0
                for t in range(NT):
                    mtot = m_lo[t] + m_hi[t]
                    ad_t = wk2.tile([P, HEADS], BF16, tag="ad")
                    nc.sync.dma_start(ad_t[:], ad_local[t * P:(t + 1) * P, :])
                    acc = psA.tile([P, DA], F32, space="PSUM", tag="acc")
                    jglob = 0
                    for half, mh in ((0, m_lo[t]), (1, m_hi[t])):
                      tbl = hh_lo if half == 0 else hh_hi
                      j = 0
                      while j < mh:
                        kb = min(KB, mh - j)
                        co = off + j
                        g4 = gp.tile([P, KB * DG], BF16, tag="hhg")
                        adg4 = psB.tile([P, KB * HEADS], F32, space="PSUM",
                                        tag="adg")
                        nc.gpsimd.dma_gather(
                            out_ap=g4[:, 0:kb * DG].rearrange(
                                "p (q d) -> p q d", q=kb),
                            in_ap=tbl[:, :],
                            idxs_ap=idx_all[:, co * 8:(co + kb) * 8],
                            num_idxs=kb * P, num_idxs_reg=kb * P,
                            elem_size=DG)
                        oh_all = wk.tile([P, KB * P], BF16, tag="ohall")
                        for q in range(kb):
                            nc.vector.tensor_tensor(
                                out=oh_all[:, q * P:(q + 1) * P],
                                in0=dst_all[:, co + q:co + q + 1].to_broadcast(
                                    [P, P]),
                                in1=iota_b[:], op=mybir.AluOpType.is_equal)
                        for q in range(kb):
                            trp = psC.tile([P, P], BF16, space="PSUM",
                                           tag="trpb")
                            nc.tensor.transpose(
                                out=trp[:], in_=oh_all[:, q * P:(q + 1) * P],
                                identity=ident[:])
                            ohT = wk.tile([P, P], BF16, tag=f"ohT{q % 2}")
                            if q % 2 == 0:
                                nc.vector.tensor_copy(ohT[:], trp[:])
                            else:
                                nc.scalar.activation(
                                    ohT[:], trp[:],
                                    mybir.ActivationFunctionType.Copy)
                            nc.tensor.matmul(
                                adg4[:, q * HEADS:(q + 1) * HEADS], lhsT=ohT[:],
                                rhs=ad_t[:], start=True, stop=True)
                        rhs4 = gp.tile([P, KB * DA], BF16, tag="rhs")
                        e4 = wk.tile([P, KB * HEADS], F32, tag="e")
                        # e = as_g + ad_g (batched over the kb gathers)
                        nc.vector.tensor_add(
                            e4[:, 0:kb * HEADS].rearrange(
                                "p (q h) -> p q h", q=kb),
                            g4[:, 0:kb * DG].rearrange(
                                "p (q d) -> p q d", q=kb)[:, :, DH:DA],
                            adg4[:, 0:kb * HEADS].rearrange(
                                "p (q h) -> p q h", q=kb))
                        nc.vector.scalar_tensor_tensor(
                            out=e4[:, 0:kb * HEADS], in0=e4[:, 0:kb * HEADS],
                            scalar=NEG, in1=e4[:, 0:kb * HEADS],
                            op0=mybir.AluOpType.mult, op1=mybir.AluOpType.max)
                        nc.scalar.activation(
                            rhs4[:, 0:kb * DA].rearrange(
                                "p (q d) -> p q d", q=kb)[:, :, DH:DA],
                            e4[:, 0:kb * HEADS].rearrange(
                                "p (q h) -> p q h", q=kb),
                            mybir.ActivationFunctionType.Exp)
                        nc.vector.tensor_mul(
                            rhs4[:, 0:kb * DA].rearrange(
                                "p (q d) -> p q d", q=kb)[:, :, 0:DH].rearrange(
                                "p q (h c) -> p q h c", h=HEADS),
                            g4[:, 0:kb * DG].rearrange(
                                "p (q d) -> p q d", q=kb)[:, :, 0:DH].rearrange(
                                "p q (h c) -> p q h c", h=HEADS),
                            rhs4[:, 0:kb * DA].rearrange(
                                "p (q d) -> p q d", q=kb)[:, :, DH:DA][
                                :, :, :, None].to_broadcast(
                                [P, kb, HEADS, HID]))
                        for q in range(kb):
                            nc.tensor.matmul(
                                acc[:], lhsT=oh_all[:, q * P:(q + 1) * P],
                                rhs=rhs4[:, q * DA:(q + 1) * DA],
                                start=(jglob + q == 0),
                                stop=(jglob + q == mtot - 1))
                        j += kb
                        jglob += kb
                      off += mh
                    # epilogue: out' = elu(num/den + bias) + 1
                    #         = max(h0,0) + exp(min(h0,0))
                    inv_t = wk2.tile([P, HEADS], F32, tag="inv")
                    nc.vector.reciprocal(inv_t[:], acc[:, DH:DA])
                    h0 = wk2.tile([P, DH], F32, tag="h0")
                    nc.vector.tensor_mul(
                        h0[:].rearrange("p (h c) -> p h c", h=HEADS),
                        acc[:, 0:DH].rearrange("p (h c) -> p h c", h=HEADS),
                        inv_t[:, :, None].to_broadcast([P, HEADS, HID]))
                    nc.vector.tensor_add(h0[:], h0[:], bias_t[l][:])
                    tm = wk2.tile([P, DH], F32, tag="tm")
                    nc.vector.tensor_tensor(out=tm[:], in0=h0[:],
                                            in1=zero_dh[:],
                                            op=mybir.AluOpType.min)
                    nc.scalar.activation(tm[:], tm[:],
                                         mybir.ActivationFunctionType.Exp)
                    out_t = wk2.tile([P, DH], F32 if last else BF16,
                                     tag="hout" if last else "houtb")
                    nc.vector.scalar_tensor_tensor(
                        out=out_t[:], in0=h0[:], scalar=0.0, in1=tm[:],
                        op0=mybir.AluOpType.max, op1=mybir.AluOpType.add)
                    if not last:
                        node_tile(l + 1, t, out_t)
                        if t == TA - 1:
                            ag_chunk(l + 1, 0)
                        elif t == NT - 1:
                            ag_chunk(l + 1, 1)
                    else:
                        gcol = wk2.tile([P, 1], F32, tag="gcol")
                        nc.sync.dma_start(gcol[:], bshift[t * P:(t + 1) * P, None])
                        ohp = wk2.tile([P, P], F32, tag="ohp")
                        nc.vector.tensor_tensor(
                            out=ohp[:], in0=gcol[:, 0:1].to_broadcast([P, P]),
                            in1=iota_f[:], op=mybir.AluOpType.is_equal)
                        nc.tensor.matmul(pool_ps[:], lhsT=ohp[:], rhs=out_t[:],
                                         start=(t == 0), stop=(t == NT - 1))

                if last:
                    # local z = pool @ lin_w, scatter into z_loc, AllReduce
                    # the tiny [GP,1] vector, then y = invcnt*z + lin_b_fold.
                    zt = wk2.tile([P, 1], F32, tag="zero1")
                    nc.gpsimd.memset(zt[:], 0.0)
                    for b in range(GP // P):
                        nc.sync.dma_start(z_loc[b * P:(b + 1) * P, :], zt[:])
                    pm = wk2.tile([P, DH], F32, tag="poolw")
                    nc.vector.tensor_mul(pm[:], pool_ps[:], linw_t[:])
                    zv = wk2.tile([P, 1], F32, tag="zv")
                    nc.vector.reduce_sum(zv[:], pm[:], axis=mybir.AxisListType.X)
                    nc.gpsimd.indirect_dma_start(
                        out=z_loc[:, :],
                        out_offset=bass.IndirectOffsetOnAxis(
                            ap=gslot_t[:, 0:1], axis=0),
                        in_=zv[:, :], in_offset=None,
                        bounds_check=GP - 1, oob_is_err=False)

            # ---------------- run the layers
            for _rep in range(reps):
                for nt in range(NT):
                    node_tile(0, nt, None)
                    if nt == TA - 1:
                        ag_chunk(0, 0)
                    elif nt == NT - 1:
                        ag_chunk(0, 1)
                for l in range(3):
                    edge_phase(l)

                if no_cc:
                    nc.sync.dma_start(z_sum[:, :], z_loc[:, :])
                else:
                    nc.gpsimd.collective_compute(
                        "AllReduce", mybir.AluOpType.add,
                        ins=[z_loc[:, :].opt()], outs=[z_sum[:, :].opt()],
                        replica_groups=[list(range(ncores))])

            # final: y = invcnt * z_sum + lin_b_fold
            for b in range(GP // P):
                pt = wk2.tile([P, 1], F32, tag="zsum_t")
                nc.sync.dma_start(pt[:], z_sum[b * P:(b + 1) * P, :])
                ic = wk2.tile([P, 1], F32, tag="ic")
                nc.sync.dma_start(ic[:], invcnt[b * P:(b + 1) * P, :])
                rs = wk2.tile([P, 1], F32, tag="rs")
                nc.vector.tensor_mul(rs[:], pt[:], ic[:])
                nc.vector.tensor_add(rs[:], rs[:], linb_t[:])
                nc.sync.dma_start(y[b * P:(b + 1) * P, :], rs[:])

    nc.compile()
    return nc


# ------------------------------------------------------------------- runner

class SpmdRunner:
    def __init__(self, nc, n_cores):
        import jax
        from jax.sharding import Mesh, PartitionSpec
        from jax.experimental.shard_map import shard_map
        from concourse.bass2jax import (
            _bass_exec_p, install_neuronx_cc_hook, partition_id_tensor)
        self.jax = jax
        install_neuronx_cc_hook()
        self.nc = nc
        self.n_cores = n_cores
        partition_name = (nc.partition_id_tensor.name
                          if nc.partition_id_tensor else None)
        in_names, out_names, out_avals, zero_outs = [], [], [], []
        for alloc in nc.m.functions[0].allocations:
            if not isinstance(alloc, mybir.MemoryLocationSet):
                continue
            name = alloc.memorylocations[0].name
            if alloc.kind == "ExternalInput":
                if name != partition_name and name != (
                        nc.dbg_addr.name if nc.dbg_addr else None):
                    in_names.append(name)
            elif alloc.kind == "ExternalOutput":
                out_names.append(name)
                shape = tuple(alloc.tensor_shape)
                dtype = mybir.dt.np(alloc.dtype)
                out_avals.append(jax.core.ShapedArray(shape, dtype))
                zero_outs.append(np.zeros(shape, dtype))
        self.in_names, self.out_names = in_names, out_names
        self.out_avals, self.zero_outs = out_avals, zero_outs
        n_params = len(in_names)
        all_in_names = list(in_names) + list(out_names)
        has_dbg = nc.dbg_addr is not None
        if has_dbg:
            all_in_names.append(nc.dbg_addr.name)
        if partition_name is not None:
            all_in_names.append(partition_name)

        def _body(*args):
            operands = list(args)
            if has_dbg:
                operands.append(jax.numpy.zeros((1, 2), jax.numpy.uint32))
            if partition_name is not None:
                operands.append(partition_id_tensor())
            outs = _bass_exec_p.bind(
                *operands, out_avals=tuple(out_avals),
                in_names=tuple(all_in_names), out_names=tuple(out_names),
                lowering_input_output_aliases=(),
                sim_require_finite=False, sim_require_nnan=False, nc=nc)
            return tuple(outs)

        devices = jax.devices()[:n_cores]
        assert len(devices) == n_cores
        mesh = Mesh(np.asarray(devices), ("core",))
        in_specs = (PartitionSpec("core"),) * (n_params + len(out_names))
        out_specs = (PartitionSpec("core"),) * len(out_names)
        self.fn = jax.jit(
            shard_map(_body, mesh=mesh, in_specs=in_specs,
                      out_specs=out_specs, check_rep=False),
            keep_unused=True)

    def prepare(self, in_maps):
        per_core = [[np.ascontiguousarray(m[nm]) for nm in self.in_names]
                    for m in in_maps]
        concat_in = [
            np.concatenate([per_core[c][i] for c in range(self.n_cores)], axis=0)
            for i in range(len(self.in_names))]
        concat_zero = [
            np.zeros((self.n_cores * z.shape[0], *z.shape[1:]), z.dtype)
            for z in self.zero_outs]
        args = [self.jax.device_put(a) for a in concat_in + concat_zero]
        for a in args:
            a.block_until_ready()
        return args

    def run(self, args):
        outs = self.fn(*args)
        self.jax.block_until_ready(outs)
        return outs

    def results(self, outs):
        res = []
        for c in range(self.n_cores):
            m = {}
            for i, nm in enumerate(self.out_names):
                m[nm] = np.asarray(outs[i]).reshape(
                    self.n_cores, *self.out_avals[i].shape)[c]
            res.append(m)
        return res


# -------------------------------------------------------------------- kernel

_CACHE = {}

N_FULL, E_FULL, G_FULL, NCORES = 50000, 800000, 512, 8
USE_F32R = False


def kernel(x, edge_index, batch,
           W0, a_src0, a_dst0, bias0,
           W1, a_src1, a_dst1, bias1,
           W2, a_src2, a_dst2, bias2,
           lin_w, lin_b):
    x = np.asarray(x, np.float32)
    edge_index = np.asarray(edge_index, np.int64)
    batch = np.asarray(batch, np.int64)
    N, E, G = x.shape[0], edge_index.shape[1], G_FULL

    cfg, in_maps = host_prep(
        x, edge_index, batch,
        [np.asarray(W0, np.float32), np.asarray(W1, np.float32),
         np.asarray(W2, np.float32)],
        [np.asarray(a_src0, np.float32), np.asarray(a_src1, np.float32),
         np.asarray(a_src2, np.float32)],
        [np.asarray(a_dst0, np.float32), np.asarray(a_dst1, np.float32),
         np.asarray(a_dst2, np.float32)],
        [np.asarray(bias0, np.float32), np.asarray(bias1, np.float32),
         np.asarray(bias2, np.float32)],
        np.asarray(lin_w, np.float32), np.asarray(lin_b, np.float32),
        N, E, G, NCORES)

    key = (cfg["NL"], cfg["NT"], cfg["GP"], cfg["m_lo"], cfg["m_hi"],
           cfg["SM"], cfg["ncores"], USE_F32R)
    if key not in _CACHE:
        nc = build_program(cfg, use_f32r=USE_F32R)
        _CACHE[key] = (nc, SpmdRunner(nc, NCORES))
    nc, runner = _CACHE[key]

    args = runner.prepare(in_maps)
    outs = runner.run(args)
    res = runner.results(outs)
    return res[0]["y"][:G].astype(np.float32)
